# revision 40
# baseline (speedup 1.0000x reference)
"""ConfidenceGuidedGate (MoE routing) Trainium2 kernel, final (v14).

Computes, for x [N=16384, D=4096], W [E=128, D], b [E]:
    logits = x @ W.T + b; conf = sigmoid(logits); top_k(conf, k=2)

Data-parallel over 8 NeuronCores (2048 tokens/core), W/b replicated.
Baseline v3 184.2us -> this kernel ~125-131us (HW run variance). Key
moves, each verified by perfetto/ntff trace deltas:
  - x is transposed on the HOST per shard (xt = x_shard.T): kills the
    512 PE transposes (~55us of tensor time; transpose-mode streams at
    the un-boosted 1.2 GHz clock and never engages the HAM boost).
    Tensor drops from 163us busy to ~95us and DMA becomes the roofline.
  - DMA efficiency scales with transfer size (341 GB/s at 1 MB vs ~300
    at 256 KB): x streams as 1 MB super-DMAs covering 4 contraction
    chunks of one token group each (3D access pattern, 2 KiB runs).
    ~5 in flight; deeper prefetch measurably REDUCES aggregate rate.
  - W.T ships as ONE 2 MB f32 tensor in chunk-major layout ([p, c*E+e],
    16 KiB runs); the exact tf32 split (wtr + wtlo) is computed on-chip
    per 4-chunk piece (scalar f32r rounding copy + vector residual sub,
    the same ops the x split uses). Split ops for pieces 1..7 sit
    INSIDE group 0's chunk loop so the in-order scalar queue never
    head-of-line blocks x splits.
  - PE warmup uses real f32r matmuls, NOT transposes (transpose-mode
    does not count as PE-busy for the HAM clock gate): 2.4 GHz unlocks
    at ~11us instead of ~22-27us, halving ramp matmul cost.
  - scalar/vector tf32-split ops run 1024 wide (2 chunks per op).
Numerics: 3 f32r matmuls per (group, chunk) — xr*wr + xe*wr + xr*wl,
all products tf32 x tf32 = exact in fp32 PSUM. This is REQUIRED for
index-exact top-2: fp64 host simulation on the fixed inputs shows the
2-matmul scheme (x @ tf32(W.T)) flips 13/32768 indices; 3mm error is
2.1e-7 vs a 1.04e-5 min gap between ranks 2 and 3 (50x margin).
Structure: 4 token groups of 512 (one PSUM logits.T bank each),
group-outer chunk-inner loop. Epilogue per group (bias activation, PE
back-transpose, DVE Max8/MaxIndex top-2, sigmoid on the 2 winners) is
deferred into the next group's chunk loop; results collect in one
partition-major SBUF tile written by a single 256B-per-partition DMA
(host undoes the layout).
"""

import numpy as np

import concourse.bass as bass
import concourse.mybir as mybir
from concourse.bass_utils import run_bass_kernel_spmd
from concourse.masks import make_identity
from concourse.tile import TileContext
from concourse.vector_clock import ScopedClock

N, D, E, K = 16384, 4096, 128, 2
N_CORES = 8
N_TOK = N // N_CORES          # 2048 tokens per core
P = 128
N_CHUNKS = D // P             # 32 contraction chunks
N_SLABS = N_TOK // P          # 16

F32 = mybir.dt.float32
F32R = mybir.dt.float32r
U32 = mybir.dt.uint32

GROUPS = [(0, 512), (512, 512), (1024, 512), (1536, 512)]
GW = 512                      # group width (tokens)
# super-DMA chunk counts per group
XPAT0 = [4, 4, 4, 4, 4, 4, 4, 4]
XPATN = [4, 4, 4, 4, 4, 4, 4, 4]
SW = 2                        # chunks per split op (1024-wide scalar/vector)
WPC = 4                       # weight chunks per piece (split granularity)
N_WP = N_CHUNKS // WPC        # 8 weight pieces
XPF = 4                       # super-DMA prefetch depth
WARMUP_T = 22                 # dummy 512-wide matmuls: enough to bridge the
                              # DMA spin-up so HAM latches 2.4 GHz before the
                              # first real matmul and never resets (8 cold
                              # mms trip the gate at ~3.4us, the rest run warm)

MAX_WAITS = 1


class PatchedTileContext(TileContext):
    """TileContext capping per-instruction sem waits to what walrus codegen
    accepts: excess waits hoisted onto same-engine NOPs (engines are
    in-order, so an earlier same-engine wait is semantics-preserving)."""

    _nop_ctr = 0

    def _add_instruction(self, inst):
        si = inst.sync_info
        if (
            si is not None
            and len(si.on_wait) > MAX_WAITS
            and inst.engine != mybir.EngineType.Unassigned
        ):
            waits = list(si.on_wait)
            keep = waits[:MAX_WAITS]
            rest = waits[MAX_WAITS:]
            while rest:
                PatchedTileContext._nop_ctr += 1
                nop = mybir.InstNoOp(
                    name=f"I-xw-{PatchedTileContext._nop_ctr}", ins=[], outs=[]
                )
                nop.engine = inst.engine
                nop.sync_info = mybir.SyncInfo(
                    on_wait=rest[:MAX_WAITS], on_update=[]
                )
                super()._add_instruction(nop)
                rest = rest[MAX_WAITS:]
            si.on_wait = keep
        super()._add_instruction(inst)

    def _drain_and_barrier(self, tick_clock, wait_clock):
        drain_inst = self.nc.sync.drain()
        wait_clock.add_sem_waits(
            drain_inst.ins, ScopedClock({None: tick_clock.global_clock})
        )
        mi = drain_inst.ins
        si = mi.sync_info
        if si is not None and len(si.on_wait) > MAX_WAITS:
            waits = list(si.on_wait)
            si.on_wait = waits[:MAX_WAITS]
            rest = waits[MAX_WAITS:]
            while rest:
                d2 = self.nc.sync.drain()
                d2.ins.sync_info = mybir.SyncInfo(
                    on_wait=rest[:MAX_WAITS], on_update=[]
                )
                rest = rest[MAX_WAITS:]
        self.nc.all_engine_barrier()
        assert self.sems is not None
        popped = self.nc._tile_sem_poison_stack.pop()
        assert popped is self._sem_poison
        self.nc.clear_and_free_semaphores(list(self.sems.allocated().values()))
        self.nc.all_engine_barrier()


def build_kernel() -> bass.Bass:
    nc = bass.Bass("TRN2", target_bir_lowering=False, debug=False)

    # host-transposed x shard: xt = x_shard.T, [D, N_TOK]
    xt_d = nc.declare_dram_parameter("xt", [D, N_TOK], F32, isOutput=False)
    # host-prepared W.T, chunk-major per partition: wt[p, c*E+e] =
    # W.T[c*128+p, e] -> 16 KiB contiguous per partition line (fast DMA);
    # plain f32, the exact tf32 split happens on-chip
    wt_d = nc.declare_dram_parameter(
        "wt", [P, N_CHUNKS * E], F32, isOutput=False
    )
    b_d = nc.declare_dram_parameter("b", [E], F32, isOutput=False)
    # combined output, partition-major: out[p, slab, 0:2]=top2 sigmoid vals,
    # out[p, slab, 2:4]=top2 indices (uint32 bits); token = slab*128 + p
    out_d = nc.declare_dram_parameter(
        "out", [P, N_SLABS * 2 * K], F32, isOutput=True
    )

    with PatchedTileContext(nc) as tc:
        with (
            tc.tile_pool(name="const", bufs=1) as const_pool,
            tc.tile_pool(name="wt", bufs=1) as wt_pool,
            tc.tile_pool(name="xt", bufs=5) as xt_pool,
            tc.tile_pool(name="xs", bufs=6) as xs_pool,
            tc.tile_pool(name="lsb", bufs=2) as lsb_pool,
            tc.tile_pool(name="top", bufs=6) as top_pool,
            tc.tile_pool(name="pt", bufs=2, space="PSUM") as psum_t,
            tc.tile_pool(name="pw", bufs=1, space="PSUM") as psum_w,
            tc.tile_pool(name="pl", bufs=2, space="PSUM") as psum_l,
        ):
            # PE warmup: dummy f32r matmuls (transpose-mode does NOT count
            # as PE-busy for the HAM clock gate; matmuls do) release the
            # 2.4 GHz clock before the first real matmuls arrive
            wsrc = const_pool.tile([P, GW], F32)
            nc.gpsimd.memset(wsrc[:], 1.0)
            warm = psum_w.tile([P, GW], F32)
            wsrc_r = wsrc[:].bitcast(F32R)
            for _ in range(WARMUP_T):
                nc.tensor.matmul(
                    warm[:], wsrc_r[:, :P], wsrc_r, start=True, stop=True
                )

            ident = const_pool.tile([P, P], F32)
            make_identity(nc, ident[:])

            # combined output accumulator (written by every epilogue)
            out_sb = const_pool.tile([P, N_SLABS * 2 * K], F32)
            out_u32 = out_sb[:].bitcast(U32)

            # --- weights: one f32 stream, split to tf32 pair on-chip -------
            wt_sb = wt_pool.tile([P, N_CHUNKS * E], F32)
            wtr_all = wt_pool.tile([P, N_CHUNKS * E], F32R)
            wtlo_all = wt_pool.tile([P, N_CHUNKS * E], F32R)

            def issue_wt_dma(c0, c1, eng=None):
                (eng or nc.sync).dma_start(
                    out=wt_sb[:, c0 * E : c1 * E],
                    in_=wt_d[:, c0 * E : c1 * E],
                )

            def split_wt_piece(q):
                s = slice(q * WPC * E, (q + 1) * WPC * E)
                nc.scalar.copy(wtr_all[:, s], wt_sb[:, s])
                nc.vector.tensor_sub(
                    wtlo_all[:, s], wt_sb[:, s], wtr_all[:, s].bitcast(F32)
                )

            # x super-DMAs: linear list of (group, chunk_lo, n_chunks)
            xplan = []
            xidx = {}  # (g, chunk) -> linear super-tile index
            for g, (t0, _) in enumerate(GROUPS):
                c0 = 0
                for nch in (XPAT0 if g == 0 else XPATN):
                    for c in range(c0, c0 + nch):
                        xidx[(g, c)] = len(xplan)
                    xplan.append((g, c0, nch))
                    c0 += nch
                assert c0 == N_CHUNKS
            # chunk -> (tile, col offset) map, filled as DMAs are issued
            x_tiles = {}
            x_issued = [0]

            def issue_x(upto):
                while x_issued[0] < min(upto, len(xplan)):
                    g, c0, nch = xplan[x_issued[0]]
                    t0 = GROUPS[g][0]
                    t = xt_pool.tile([P, nch * GW], F32, tag="xt")
                    nc.sync.dma_start(
                        out=t[:].rearrange("p (c t) -> p c t", t=GW),
                        in_=xt_d.rearrange("(c p) t -> p c t", p=P)[
                            :, c0 : c0 + nch, t0 : t0 + GW
                        ],
                    )
                    for c in range(c0, c0 + nch):
                        x_tiles[(g, c)] = (t, (c - c0) * GW)
                    x_issued[0] += 1

            # ramp: small first weight DMA unblocks matmul chunk 0 fast,
            # the rest of W.T lands in two mid-size long-run transfers
            # interleaved so they don't head-of-line block early x tiles
            issue_wt_dma(0, WPC)
            issue_x(1)
            issue_wt_dma(WPC, 4 * WPC)
            issue_x(3)
            issue_wt_dma(4 * WPC, N_CHUNKS)
            issue_x(XPF)

            b_sb = const_pool.tile([P, 1], F32)
            nc.sync.dma_start(out=b_sb[:], in_=b_d[:])

            split_wt_piece(0)

            # --- epilogue (deferred into the next group's chunk loop) -------
            def run_epilogue(t0, ntok, pl):
                nslab = ntok // P
                lsb = lsb_pool.tile([P, GW], F32, tag="lsb")
                nc.scalar.activation(
                    lsb[:, :ntok], pl[:],
                    mybir.ActivationFunctionType.Identity,
                    bias=b_sb[:], scale=1.0,
                )
                ptb = psum_t.tile([P, ntok], F32, tag="ptb")
                for j in range(nslab):
                    nc.tensor.transpose(
                        ptb[:, j * P : (j + 1) * P],
                        lsb[:, j * P : (j + 1) * P],
                        ident[:],
                    )
                for j in range(nslab):
                    js = t0 // P + j  # global slab index
                    pv = ptb[:, j * P : (j + 1) * P]
                    vals8 = top_pool.tile([P, 8], F32, tag="v8")
                    idx8 = top_pool.tile([P, 8], U32, tag="i8")
                    nc.vector.max(vals8[:], pv)
                    nc.vector.max_index(idx8[:], vals8[:], pv)
                    nc.scalar.activation(
                        out_sb[:, js * 4 : js * 4 + K],
                        vals8[:, :K],
                        mybir.ActivationFunctionType.Sigmoid,
                    )
                    nc.vector.tensor_copy(
                        out_u32[:, js * 4 + K : js * 4 + 2 * K], idx8[:, :K]
                    )

            # --- main pipeline ----------------------------------------------
            deferred = None
            xu = 0  # linear super-DMA consumption counter
            for g, (t0, ntok) in enumerate(GROUPS):
                pl = psum_l.tile([P, ntok], F32, tag="pl")
                xts = {}
                for c in range(N_CHUNKS):
                    if c == 4 and deferred is not None:
                        run_epilogue(*deferred)
                        deferred = None
                    # on-chip W split for pieces 1..7, inside group 0's loop
                    # (their wt DMAs landed during the ramp; placing the ops
                    # here keeps them from head-of-line blocking x splits)
                    if g == 0 and c >= 2 and c % WPC == 2 and c // WPC < N_WP - 1:
                        split_wt_piece(c // WPC + 1)
                    if c % SW == 0:
                        # 1024-wide tf32 split covering chunks c, c+1
                        tile, off = x_tiles.pop((g, c))
                        tile2, off2 = x_tiles.pop((g, c + 1))
                        assert tile2 is tile and off2 == off + GW
                        xv = tile[:, off : off + SW * GW]
                        xtr = xs_pool.tile([P, SW * GW], F32R, tag="xtr")
                        xte = xs_pool.tile([P, SW * GW], F32R, tag="xte")
                        nc.scalar.copy(xtr[:], xv)
                        nc.vector.tensor_sub(xte[:], xv, xtr[:].bitcast(F32))
                        xts[c] = (xtr, 0)
                        xts[c + 1] = (xtr, GW)
                        xts[(c, "e")] = (xte, 0)
                        xts[(c + 1, "e")] = (xte, GW)
                        # keep the super-DMA stream XPF tiles ahead of use
                        issue_x(xidx[(g, c)] + 1 + XPF)
                    xtr, xo = xts[c]
                    xte, _ = xts[(c, "e")]
                    wr = wtr_all[:, c * P : (c + 1) * P]
                    wl = wtlo_all[:, c * P : (c + 1) * P]
                    nc.tensor.matmul(
                        pl[:], wr, xtr[:, xo : xo + GW],
                        start=(c == 0), stop=False,
                    )
                    nc.tensor.matmul(
                        pl[:], wr, xte[:, xo : xo + GW], start=False, stop=False
                    )
                    nc.tensor.matmul(
                        pl[:], wl, xtr[:, xo : xo + GW],
                        start=False, stop=(c == N_CHUNKS - 1),
                    )

                if g == len(GROUPS) - 1:
                    run_epilogue(t0, ntok, pl)
                else:
                    deferred = (t0, ntok, pl)

            # single dense output DMA: 256B contiguous per partition
            nc.sync.dma_start(out=out_d[:, :], in_=out_sb[:])

    return nc


_NC_CACHE = None


def _get_nc():
    global _NC_CACHE
    if _NC_CACHE is None:
        _NC_CACHE = build_kernel()
    return _NC_CACHE


def _unpack(out: np.ndarray):
    """[P, 16*4] partition-major -> (vals [2048,2] f32, idx [2048,2] i32)."""
    a = out.reshape(P, N_SLABS, 2 * K).transpose(1, 0, 2).reshape(N_TOK, 2 * K)
    vals = a[:, :K].copy()
    idx = a[:, K : 2 * K].copy().view(np.int32)
    return vals, idx


def run_sharded(x, WT, b, trace=False, **kw):
    nc = _get_nc()
    # chunk-major weight layout: wt[p, c*E+e] = WT[c*128+p, e]
    wt_host = np.ascontiguousarray(
        WT.reshape(N_CHUNKS, P, E).transpose(1, 0, 2).reshape(P, N_CHUNKS * E)
    )
    in_maps = []
    for i in range(N_CORES):
        in_maps.append(
            {
                "xt": np.ascontiguousarray(
                    x[i * N_TOK : (i + 1) * N_TOK].T
                ),
                "wt": wt_host,
                "b": b,
            }
        )
    return run_bass_kernel_spmd(
        nc, in_maps, core_ids=list(range(N_CORES)), trace=trace, **kw
    )


def kernel(x, W, b):
    x = np.asarray(x, dtype=np.float32)
    W = np.asarray(W, dtype=np.float32)
    b = np.asarray(b, dtype=np.float32)
    WT = np.ascontiguousarray(W.T)
    res = run_sharded(x, WT, b)
    vals_l, idx_l = [], []
    for r in res.results:
        v, i = _unpack(r["out"])
        vals_l.append(v)
        idx_l.append(i)
    return np.concatenate(vals_l, axis=0), np.concatenate(idx_l, axis=0)


# revision 42
# speedup vs baseline: 1.0634x; 1.0634x over previous
"""ConfidenceGuidedGate (MoE routing) Trainium2 kernel, final (v14).

Computes, for x [N=16384, D=4096], W [E=128, D], b [E]:
    logits = x @ W.T + b; conf = sigmoid(logits); top_k(conf, k=2)

Data-parallel over 8 NeuronCores (2048 tokens/core), W/b replicated.
Baseline v3 184.2us -> this kernel ~125-131us (HW run variance). Key
moves, each verified by perfetto/ntff trace deltas:
  - x is transposed on the HOST per shard (xt = x_shard.T): kills the
    512 PE transposes (~55us of tensor time; transpose-mode streams at
    the un-boosted 1.2 GHz clock and never engages the HAM boost).
    Tensor drops from 163us busy to ~95us and DMA becomes the roofline.
  - DMA efficiency scales with transfer size (341 GB/s at 1 MB vs ~300
    at 256 KB): x streams as 1 MB super-DMAs covering 4 contraction
    chunks of one token group each (3D access pattern, 2 KiB runs).
    ~5 in flight; deeper prefetch measurably REDUCES aggregate rate.
  - W.T ships as ONE 2 MB f32 tensor in chunk-major layout ([p, c*E+e],
    16 KiB runs); the exact tf32 split (wtr + wtlo) is computed on-chip
    per 4-chunk piece (scalar f32r rounding copy + vector residual sub,
    the same ops the x split uses). Split ops for pieces 1..7 sit
    INSIDE group 0's chunk loop so the in-order scalar queue never
    head-of-line blocks x splits.
  - PE warmup uses real f32r matmuls, NOT transposes (transpose-mode
    does not count as PE-busy for the HAM clock gate): 2.4 GHz unlocks
    at ~11us instead of ~22-27us, halving ramp matmul cost.
  - scalar/vector tf32-split ops run 1024 wide (2 chunks per op).
Numerics: 3 f32r matmuls per (group, chunk) — xr*wr + xe*wr + xr*wl,
all products tf32 x tf32 = exact in fp32 PSUM. This is REQUIRED for
index-exact top-2: fp64 host simulation on the fixed inputs shows the
2-matmul scheme (x @ tf32(W.T)) flips 13/32768 indices; 3mm error is
2.1e-7 vs a 1.04e-5 min gap between ranks 2 and 3 (50x margin).
Structure: 4 token groups of 512 (one PSUM logits.T bank each),
group-outer chunk-inner loop. Epilogue per group (bias activation, PE
back-transpose, DVE Max8/MaxIndex top-2, sigmoid on the 2 winners) is
deferred into the next group's chunk loop; results collect in one
partition-major SBUF tile written by a single 256B-per-partition DMA
(host undoes the layout).
"""

import numpy as np

import concourse.bass as bass
import concourse.mybir as mybir
from concourse.bass_utils import run_bass_kernel_spmd
from concourse.masks import make_identity
from concourse.tile import TileContext
from concourse.vector_clock import ScopedClock

N, D, E, K = 16384, 4096, 128, 2
N_CORES = 8
N_TOK = N // N_CORES          # 2048 tokens per core
P = 128
N_CHUNKS = D // P             # 32 contraction chunks
N_SLABS = N_TOK // P          # 16

F32 = mybir.dt.float32
F32R = mybir.dt.float32r
U32 = mybir.dt.uint32

GROUPS = [(0, 512), (512, 512), (1024, 512), (1536, 512)]
GW = 512                      # group width (tokens)
# super-DMA chunk counts per group
XPAT0 = [4, 4, 4, 4, 4, 4, 4, 4]
XPATN = [4, 4, 4, 4, 4, 4, 4, 4]
SW = 2                        # chunks per split op (1024-wide scalar/vector)
WPC = 4                       # weight chunks per piece (split granularity)
N_WP = N_CHUNKS // WPC        # 8 weight pieces
XPF = 6                       # super-DMA prefetch depth
WARMUP_T = 22                 # dummy 512-wide matmuls: enough to bridge the
                              # DMA spin-up so HAM latches 2.4 GHz before the
                              # first real matmul and never resets (8 cold
                              # mms trip the gate at ~3.4us, the rest run warm)

MAX_WAITS = 1


class PatchedTileContext(TileContext):
    """TileContext capping per-instruction sem waits to what walrus codegen
    accepts: excess waits hoisted onto same-engine NOPs (engines are
    in-order, so an earlier same-engine wait is semantics-preserving)."""

    _nop_ctr = 0

    def _add_instruction(self, inst):
        si = inst.sync_info
        if (
            si is not None
            and len(si.on_wait) > MAX_WAITS
            and inst.engine != mybir.EngineType.Unassigned
        ):
            waits = list(si.on_wait)
            keep = waits[:MAX_WAITS]
            rest = waits[MAX_WAITS:]
            while rest:
                PatchedTileContext._nop_ctr += 1
                nop = mybir.InstNoOp(
                    name=f"I-xw-{PatchedTileContext._nop_ctr}", ins=[], outs=[]
                )
                nop.engine = inst.engine
                nop.sync_info = mybir.SyncInfo(
                    on_wait=rest[:MAX_WAITS], on_update=[]
                )
                super()._add_instruction(nop)
                rest = rest[MAX_WAITS:]
            si.on_wait = keep
        super()._add_instruction(inst)

    def _drain_and_barrier(self, tick_clock, wait_clock):
        drain_inst = self.nc.sync.drain()
        wait_clock.add_sem_waits(
            drain_inst.ins, ScopedClock({None: tick_clock.global_clock})
        )
        mi = drain_inst.ins
        si = mi.sync_info
        if si is not None and len(si.on_wait) > MAX_WAITS:
            waits = list(si.on_wait)
            si.on_wait = waits[:MAX_WAITS]
            rest = waits[MAX_WAITS:]
            while rest:
                d2 = self.nc.sync.drain()
                d2.ins.sync_info = mybir.SyncInfo(
                    on_wait=rest[:MAX_WAITS], on_update=[]
                )
                rest = rest[MAX_WAITS:]
        self.nc.all_engine_barrier()
        assert self.sems is not None
        popped = self.nc._tile_sem_poison_stack.pop()
        assert popped is self._sem_poison
        self.nc.clear_and_free_semaphores(list(self.sems.allocated().values()))
        self.nc.all_engine_barrier()


def build_kernel() -> bass.Bass:
    nc = bass.Bass("TRN2", target_bir_lowering=False, debug=False)

    # host-transposed x shard: xt = x_shard.T, [D, N_TOK]
    xt_d = nc.declare_dram_parameter("xt", [D, N_TOK], F32, isOutput=False)
    # host-prepared W.T, chunk-major per partition: wt[p, c*E+e] =
    # W.T[c*128+p, e] -> 16 KiB contiguous per partition line (fast DMA);
    # plain f32, the exact tf32 split happens on-chip
    wt_d = nc.declare_dram_parameter(
        "wt", [P, N_CHUNKS * E], F32, isOutput=False
    )
    b_d = nc.declare_dram_parameter("b", [E], F32, isOutput=False)
    # combined output, partition-major: out[p, slab, 0:2]=top2 sigmoid vals,
    # out[p, slab, 2:4]=top2 indices (uint32 bits); token = slab*128 + p
    out_d = nc.declare_dram_parameter(
        "out", [P, N_SLABS * 2 * K], F32, isOutput=True
    )

    with PatchedTileContext(nc) as tc:
        with (
            tc.tile_pool(name="const", bufs=1) as const_pool,
            tc.tile_pool(name="wt", bufs=1) as wt_pool,
            tc.tile_pool(name="xt", bufs=7) as xt_pool,
            tc.tile_pool(name="xs", bufs=6) as xs_pool,
            tc.tile_pool(name="lsb", bufs=2) as lsb_pool,
            tc.tile_pool(name="top", bufs=6) as top_pool,
            tc.tile_pool(name="pt", bufs=2, space="PSUM") as psum_t,
            tc.tile_pool(name="pw", bufs=1, space="PSUM") as psum_w,
            tc.tile_pool(name="pl", bufs=2, space="PSUM") as psum_l,
        ):
            # PE warmup: dummy f32r matmuls (transpose-mode does NOT count
            # as PE-busy for the HAM clock gate; matmuls do) release the
            # 2.4 GHz clock before the first real matmuls arrive
            wsrc = const_pool.tile([P, GW], F32)
            nc.gpsimd.memset(wsrc[:], 1.0)
            warm = psum_w.tile([P, GW], F32)
            wsrc_r = wsrc[:].bitcast(F32R)
            for _ in range(WARMUP_T):
                nc.tensor.matmul(
                    warm[:], wsrc_r[:, :P], wsrc_r, start=True, stop=True
                )

            ident = const_pool.tile([P, P], F32)
            make_identity(nc, ident[:])

            # combined output accumulator (written by every epilogue)
            out_sb = const_pool.tile([P, N_SLABS * 2 * K], F32)
            out_u32 = out_sb[:].bitcast(U32)

            # --- weights: one f32 stream, split to tf32 pair on-chip -------
            wt_sb = wt_pool.tile([P, N_CHUNKS * E], F32)
            wtr_all = wt_pool.tile([P, N_CHUNKS * E], F32R)
            wtlo_all = wt_pool.tile([P, N_CHUNKS * E], F32R)

            def issue_wt_dma(c0, c1, eng=None):
                (eng or nc.sync).dma_start(
                    out=wt_sb[:, c0 * E : c1 * E],
                    in_=wt_d[:, c0 * E : c1 * E],
                )

            def split_wt_piece(q):
                s = slice(q * WPC * E, (q + 1) * WPC * E)
                nc.scalar.copy(wtr_all[:, s], wt_sb[:, s])
                nc.vector.tensor_sub(
                    wtlo_all[:, s], wt_sb[:, s], wtr_all[:, s].bitcast(F32)
                )

            # x super-DMAs: linear list of (group, chunk_lo, n_chunks)
            xplan = []
            xidx = {}  # (g, chunk) -> linear super-tile index
            for g, (t0, _) in enumerate(GROUPS):
                c0 = 0
                for nch in (XPAT0 if g == 0 else XPATN):
                    for c in range(c0, c0 + nch):
                        xidx[(g, c)] = len(xplan)
                    xplan.append((g, c0, nch))
                    c0 += nch
                assert c0 == N_CHUNKS
            # chunk -> (tile, col offset) map, filled as DMAs are issued
            x_tiles = {}
            x_issued = [0]

            def issue_x(upto):
                while x_issued[0] < min(upto, len(xplan)):
                    g, c0, nch = xplan[x_issued[0]]
                    t0 = GROUPS[g][0]
                    t = xt_pool.tile([P, nch * GW], F32, tag="xt")
                    nc.sync.dma_start(
                        out=t[:].rearrange("p (c t) -> p c t", t=GW),
                        in_=xt_d.rearrange("(c p) t -> p c t", p=P)[
                            :, c0 : c0 + nch, t0 : t0 + GW
                        ],
                    )
                    for c in range(c0, c0 + nch):
                        x_tiles[(g, c)] = (t, (c - c0) * GW)
                    x_issued[0] += 1

            # ramp: small first weight DMA unblocks matmul chunk 0 fast,
            # the rest of W.T lands in two mid-size long-run transfers
            # interleaved so they don't head-of-line block early x tiles
            issue_wt_dma(0, WPC)
            issue_x(1)
            issue_wt_dma(WPC, 4 * WPC)
            issue_x(3)
            issue_wt_dma(4 * WPC, N_CHUNKS)
            issue_x(XPF)

            b_sb = const_pool.tile([P, 1], F32)
            nc.sync.dma_start(out=b_sb[:], in_=b_d[:])

            split_wt_piece(0)

            # --- epilogue (deferred into the next group's chunk loop) -------
            def run_epilogue(t0, ntok, pl):
                nslab = ntok // P
                lsb = lsb_pool.tile([P, GW], F32, tag="lsb")
                nc.scalar.activation(
                    lsb[:, :ntok], pl[:],
                    mybir.ActivationFunctionType.Identity,
                    bias=b_sb[:], scale=1.0,
                )
                ptb = psum_t.tile([P, ntok], F32, tag="ptb")
                for j in range(nslab):
                    nc.tensor.transpose(
                        ptb[:, j * P : (j + 1) * P],
                        lsb[:, j * P : (j + 1) * P],
                        ident[:],
                    )
                for j in range(nslab):
                    js = t0 // P + j  # global slab index
                    pv = ptb[:, j * P : (j + 1) * P]
                    vals8 = top_pool.tile([P, 8], F32, tag="v8")
                    idx8 = top_pool.tile([P, 8], U32, tag="i8")
                    nc.vector.max(vals8[:], pv)
                    nc.vector.max_index(idx8[:], vals8[:], pv)
                    nc.scalar.activation(
                        out_sb[:, js * 4 : js * 4 + K],
                        vals8[:, :K],
                        mybir.ActivationFunctionType.Sigmoid,
                    )
                    nc.vector.tensor_copy(
                        out_u32[:, js * 4 + K : js * 4 + 2 * K], idx8[:, :K]
                    )

            # --- main pipeline ----------------------------------------------
            deferred = None
            xu = 0  # linear super-DMA consumption counter
            for g, (t0, ntok) in enumerate(GROUPS):
                pl = psum_l.tile([P, ntok], F32, tag="pl")
                xts = {}
                for c in range(N_CHUNKS):
                    if c == 4 and deferred is not None:
                        run_epilogue(*deferred)
                        deferred = None
                    # on-chip W split for pieces 1..7, inside group 0's loop
                    # (their wt DMAs landed during the ramp; placing the ops
                    # here keeps them from head-of-line blocking x splits)
                    if g == 0 and c >= 2 and c % WPC == 2 and c // WPC < N_WP - 1:
                        split_wt_piece(c // WPC + 1)
                    if c % SW == 0:
                        # 1024-wide tf32 split covering chunks c, c+1
                        tile, off = x_tiles.pop((g, c))
                        tile2, off2 = x_tiles.pop((g, c + 1))
                        assert tile2 is tile and off2 == off + GW
                        xv = tile[:, off : off + SW * GW]
                        xtr = xs_pool.tile([P, SW * GW], F32R, tag="xtr")
                        xte = xs_pool.tile([P, SW * GW], F32R, tag="xte")
                        nc.scalar.copy(xtr[:], xv)
                        nc.vector.tensor_sub(xte[:], xv, xtr[:].bitcast(F32))
                        xts[c] = (xtr, 0)
                        xts[c + 1] = (xtr, GW)
                        xts[(c, "e")] = (xte, 0)
                        xts[(c + 1, "e")] = (xte, GW)
                        # keep the super-DMA stream XPF tiles ahead of use
                        issue_x(xidx[(g, c)] + 1 + XPF)
                    xtr, xo = xts[c]
                    xte, _ = xts[(c, "e")]
                    wr = wtr_all[:, c * P : (c + 1) * P]
                    wl = wtlo_all[:, c * P : (c + 1) * P]
                    nc.tensor.matmul(
                        pl[:], wr, xtr[:, xo : xo + GW],
                        start=(c == 0), stop=False,
                    )
                    nc.tensor.matmul(
                        pl[:], wr, xte[:, xo : xo + GW], start=False, stop=False
                    )
                    nc.tensor.matmul(
                        pl[:], wl, xtr[:, xo : xo + GW],
                        start=False, stop=(c == N_CHUNKS - 1),
                    )

                if g == len(GROUPS) - 1:
                    run_epilogue(t0, ntok, pl)
                else:
                    deferred = (t0, ntok, pl)

            # single dense output DMA: 256B contiguous per partition
            nc.sync.dma_start(out=out_d[:, :], in_=out_sb[:])

    return nc


_NC_CACHE = None


def _get_nc():
    global _NC_CACHE
    if _NC_CACHE is None:
        _NC_CACHE = build_kernel()
    return _NC_CACHE


def _unpack(out: np.ndarray):
    """[P, 16*4] partition-major -> (vals [2048,2] f32, idx [2048,2] i32)."""
    a = out.reshape(P, N_SLABS, 2 * K).transpose(1, 0, 2).reshape(N_TOK, 2 * K)
    vals = a[:, :K].copy()
    idx = a[:, K : 2 * K].copy().view(np.int32)
    return vals, idx


def run_sharded(x, WT, b, trace=False, **kw):
    nc = _get_nc()
    # chunk-major weight layout: wt[p, c*E+e] = WT[c*128+p, e]
    wt_host = np.ascontiguousarray(
        WT.reshape(N_CHUNKS, P, E).transpose(1, 0, 2).reshape(P, N_CHUNKS * E)
    )
    in_maps = []
    for i in range(N_CORES):
        in_maps.append(
            {
                "xt": np.ascontiguousarray(
                    x[i * N_TOK : (i + 1) * N_TOK].T
                ),
                "wt": wt_host,
                "b": b,
            }
        )
    return run_bass_kernel_spmd(
        nc, in_maps, core_ids=list(range(N_CORES)), trace=trace, **kw
    )


def kernel(x, W, b):
    x = np.asarray(x, dtype=np.float32)
    W = np.asarray(W, dtype=np.float32)
    b = np.asarray(b, dtype=np.float32)
    WT = np.ascontiguousarray(W.T)
    res = run_sharded(x, WT, b)
    vals_l, idx_l = [], []
    for r in res.results:
        v, i = _unpack(r["out"])
        vals_l.append(v)
        idx_l.append(i)
    return np.concatenate(vals_l, axis=0), np.concatenate(idx_l, axis=0)
